# revision 24
# baseline (speedup 1.0000x reference)
"""ClsbdCRF message-passing kernel for 8 Trainium2 NeuronCores.

Sharding: core i handles batch b = i//2 and image-row half i%2 (64 output
rows each, with span-2 halos sliced host-side).  Per-core SBUF layout puts
W=128 on partitions and (C, H) on the free dimension, so the 5x5 stencil
becomes partition-offset (dy) + free-offset (dx) access patterns.

Math per core (fp32):
  pl   = 1 - ent/ln(C),  ent = -sum_c x ln(x+eps)
  xp   = x * pl
  g1_d = exp(-0.5 * ||f(x) - f(x+d)||^2)            (12 taps + mirrors + center)
  g2_t = ring-max propagation of unfolded clsbd map  (24 taps + center=0)
  w_neg_t = 2*g1_t - ln(g2_t+eps)          (x5 at the end)
  w_pos_t = ln(1 - g2_t + eps)             (x-5 at the end)
  msg[c,h,w] = sum_t w_t[h,w] * xp[c, (h,w)+t]

Boundary handling: H is zero-padded host-side (feats big-padded so the
pairwise gaussian underflows to exactly 0 out of image); W taps use
restricted partition ranges with pre-zeroed destination tiles.
"""

import math

import numpy as np

B, C, H, W, D = 4, 21, 128, 128, 5
SPAN = 2
EPS = 1e-5
HP = 64          # output rows per core
HE = HP + 4      # input / clsbd row extent (halo 2 each side)
FE = HP + 8      # feats row extent (halo 4 each side)
BIGPAD = 1000.0  # feats pad value; (BIGPAD-x)^2 makes exp() underflow to 0
COMPAT_PAIR = 10.0
COMPAT_CLSBD = 5.0

RING1 = [(-1, -1), (-1, 0), (-1, 1), (0, -1), (0, 1), (1, -1), (1, 0), (1, 1)]
RING2 = [(-2, -2), (-2, -1), (-2, 0), (-2, 1), (-2, 2), (-1, -2), (-1, 2),
         (0, -2), (0, 2), (1, -2), (1, 2), (2, -2), (2, -1), (2, 0), (2, 1),
         (2, 2)]
EXP1 = [0, 0, 1, 2, 2, 0, 2, 3, 4, 5, 7, 5, 5, 6, 7, 7]
EXP2 = [0, 1, 1, 1, 2, 3, 4, 3, 4, 3, 4, 5, 6, 6, 6, 7]
# taps whose pairwise gaussian is computed directly; mirrors are shifted reads
DIRTAPS = [(dx, dy) for dx in range(-SPAN, SPAN + 1)
           for dy in range(-SPAN, SPAN + 1) if (dx, dy) > (0, 0)]
ALLTAPS = [(dx, dy) for dx in range(-SPAN, SPAN + 1)
           for dy in range(-SPAN, SPAN + 1)]

GP_NTAPS = 8
_cache = {}


def _wrange(dy):
    return max(0, -dy), W - max(0, dy)


def _build():
    import concourse.bacc as bacc
    import concourse.mybir as mybir
    from concourse.tile import TileContext

    f32 = mybir.dt.float32
    Act = mybir.ActivationFunctionType
    Alu = mybir.AluOpType

    nc = bacc.Bacc()
    x_d = nc.declare_dram_parameter("x", [W, C, HE], f32, isOutput=False)
    f_d = nc.declare_dram_parameter("f", [W, D, FE], f32, isOutput=False)
    s_d = nc.declare_dram_parameter("s", [W, HE], f32, isOutput=False)
    o_d = nc.declare_dram_parameter("out", [2, W, C, HP], f32, isOutput=True)

    DYS = [-2, -1, 1, 2]

    # Pre-TileContext constants, covered by an all-engine barrier (same
    # pattern Bass.__init__ uses) so consumers never need a sync wait.
    def _const_sbuf(name, shape, val):
        t = nc.alloc_sbuf_tensor(name, shape, f32)
        nc.gpsimd.memset(t.ap(), val)
        return t.ap()

    zt = _const_sbuf("zt_const", [W, C, HE], 0.0)
    bpad = _const_sbuf("bpad_const", [W, D, FE], BIGPAD)
    b_eps = _const_sbuf("b_eps", [W, 1], EPS)
    b_ln2 = _const_sbuf("b_ln2", [W, 1], math.log(2.0))
    b_1eps = _const_sbuf("b_1eps", [W, 1], 1.0 + EPS)
    nc.const_aps.aps[(f32, EPS)] = b_eps
    nc.all_engine_barrier()

    # taps owned end-to-end by GpSimd (ready earliest: dy=0 needs no
    # shifted xp at all). DVE keeps the rest.
    GP_TAPS = [(1, 0), (2, 0), (-1, 0), (-2, 0)][:GP_NTAPS]

    with TileContext(nc) as tc:
        with (
            tc.tile_pool(name="io", bufs=1) as io,
            tc.tile_pool(name="g1p", bufs=1) as g1p,
            tc.tile_pool(name="g2p", bufs=1) as g2p,
            tc.tile_pool(name="wp", bufs=1) as wp,
            tc.tile_pool(name="lp", bufs=3) as lp,
            tc.tile_pool(name="mp", bufs=2) as mp,
            tc.tile_pool(name="sc", bufs=1) as sc,
            tc.tile_pool(name="scr", bufs=2) as scr,
            tc.tile_pool(name="dr", bufs=1, space="DRAM") as dr,
        ):
            # ---- phase 0: all DRAM loads up front ----
            x_t = io.tile([W, C, HE], f32, tag="x")
            f_t = io.tile([W, D, FE], f32, tag="f")
            s_t = io.tile([W, HE], f32, tag="s")
            nc.sync.dma_start(out=x_t[:], in_=x_d[:])
            nc.sync.dma_start(out=f_t[:], in_=f_d[:])
            nc.sync.dma_start(out=s_t[:], in_=s_d[:])

            def _shift_load(pool, tag, shape, dram, padsrc, dy, eng=None):
                # dy-shifted copy straight from DRAM (one descriptor),
                # out-of-range partitions filled from a barrier-covered const
                eng = eng or nc.sync
                t = pool.tile(shape, f32, tag=tag)
                a, b = _wrange(dy)
                if a > 0:
                    eng.dma_start(out=t[:a], in_=padsrc[:a])
                if b < W:
                    eng.dma_start(out=t[b:], in_=padsrc[b:])
                eng.dma_start(out=t[a:b], in_=dram[a + dy:b + dy])
                return t

            f_s, s_s = {0: f_t}, {0: s_t}
            for dy in DYS:
                f_s[dy] = _shift_load(io, f"fs_{dy}", [W, D, FE], f_d, bpad,
                                      dy)
                s_s[dy] = _shift_load(io, f"ss_{dy}", [W, HE], s_d,
                                      zt[:, 0, :HE], dy)

            # ---- polarness ----
            lnx = sc.tile([W, C, HE], f32, tag="lnx")
            nc.scalar.activation(lnx[:], x_t[:], Act.Ln, bias=b_eps[:], scale=1.0)
            xl = sc.tile([W, C, HE], f32, tag="xl")
            nc.vector.tensor_mul(xl[:], x_t[:], lnx[:])
            ent = sc.tile([W, HE], f32, tag="ent")
            nc.vector.tensor_reduce(
                out=ent[:], in_=xl[:].rearrange("p c h -> p h c"),
                axis=mybir.AxisListType.X, op=Alu.add)
            pl = sc.tile([W, HE], f32, tag="pl")
            # ent holds sum_c x*ln(x+eps) = -entropy
            nc.scalar.activation(pl[:], ent[:], Act.Copy,
                                 bias=1.0, scale=1.0 / math.log(C))
            xp = io.tile([W, C, HE], f32, tag="xp")
            nc.vector.tensor_mul(
                xp[:], x_t[:], pl[:, None, :].broadcast_to((W, C, HE)))
            xp_s = {0: xp}
            for dy in DYS:
                a, b = _wrange(dy)
                t = io.tile([W, C, HE], f32, tag=f"xps_{dy}")
                if a > 0:
                    nc.sync.dma_start(out=t[:a], in_=zt[:a])
                if b < W:
                    nc.sync.dma_start(out=t[b:], in_=zt[b:])
                nc.sync.dma_start(out=t[a:b], in_=xp[a + dy:b + dy])
                xp_s[dy] = t

            # ---- pairwise gaussian (12 direct taps; value stored = 2*g1) ----
            # BIGPAD-shifted feats make out-of-image taps underflow to 0.
            g1t = {}
            g1d = {}
            for (dx, dy) in DIRTAPS:
                g1 = g1p.tile([W, HE], f32, tag=f"g1_{dx}_{dy}")
                diff = scr.tile([W, D, HE], f32, tag="diff")
                nc.gpsimd.tensor_sub(
                    diff[:], f_t[:, :, 2:2 + HE],
                    f_s[dy][:, :, 2 + dx:2 + dx + HE])
                sq = scr.tile([W, D, HE], f32, tag="sq")
                nc.scalar.square(sq[:], diff[:])
                ssum = scr.tile([W, HE], f32, tag="ssum")
                nc.vector.tensor_reduce(
                    out=ssum[:], in_=sq[:].rearrange("p d h -> p h d"),
                    axis=mybir.AxisListType.X, op=Alu.add)
                nc.scalar.activation(g1[:], ssum[:], Act.Exp,
                                     bias=b_ln2[:], scale=-0.5)
                g1t[(dx, dy)] = g1
                if dy != 0:
                    gd = dr.tile([W, HE], f32, tag=f"g1d_{dx}_{dy}")
                    nc.scalar.dma_start(out=gd[:], in_=g1[:])
                    g1d[(dx, dy)] = gd
            # dy-shifted mirror copies via DRAM roundtrip
            g1s = {}
            for (dx, dy) in DIRTAPS:
                if dy == 0:
                    g1s[(dx, dy)] = g1t[(dx, dy)]
                    continue
                a, b = _wrange(-dy)
                t = g1p.tile([W, HE], f32, tag=f"g1s_{dx}_{dy}")
                if a > 0:
                    nc.scalar.dma_start(out=t[:a], in_=zt[:a, 0, :HE])
                if b < W:
                    nc.scalar.dma_start(out=t[b:], in_=zt[b:, 0, :HE])
                nc.scalar.dma_start(out=t[a:b],
                                    in_=g1d[(dx, dy)][a - dy:b - dy])
                g1s[(dx, dy)] = t

            # ---- clsbd gaussian: ring max propagation ----
            tmp1 = [s_s[dy][:, 2 + dx:2 + dx + HP] for (dx, dy) in RING1]
            g2t = {t: tmp1[j] for j, t in enumerate(RING1)}
            for k, (dx, dy) in enumerate(RING2):
                t2 = g2p.tile([W, HP], f32, tag=f"t2_{k}")
                nc.vector.tensor_max(t2[:], tmp1[EXP1[k]], tmp1[EXP2[k]])
                nc.vector.tensor_max(
                    t2[:], t2[:], s_s[dy][:, 2 + dx:2 + dx + HP])
                g2t[(dx, dy)] = t2

            # ---- weights + products, one pass per tap ----
            # order: GpSimd-owned first, then direct DVE taps, then mirror
            # taps (their wn waits on the g1s roundtrip DMA; last keeps
            # Vector's in-order queue free of head-of-line stalls).
            NT = [t for t in ALLTAPS if t != (0, 0)]
            NT.sort(key=lambda t: (t not in GP_TAPS,
                                   not (t > (0, 0)), abs(t[1])))
            accn = io.tile([W, C, HP], f32, tag="accn")
            accp = io.tile([W, C, HP], f32, tag="accp")
            accn2 = io.tile([W, C, HP], f32, tag="accn2")
            accp2 = io.tile([W, C, HP], f32, tag="accp2")
            xp_c = xp[:, :, 2:2 + HP]
            nc.vector.tensor_scalar_mul(accn[:], xp_c, 2.0 - math.log(EPS))
            nc.vector.tensor_scalar_mul(accp[:], xp_c, math.log(1.0 + EPS))
            gp_first = [True]
            for (dx, dy) in NT:
                g2 = g2t[(dx, dy)]
                g2ap = g2[:] if hasattr(g2, "tag") else g2
                lnn = lp.tile([W, HP], f32, tag="lnn")
                nc.scalar.activation(lnn[:], g2ap, Act.Ln, bias=b_eps[:],
                                     scale=1.0)
                lnp = wp.tile([W, HP], f32, tag=f"lnp_{dx}_{dy}")
                nc.scalar.activation(lnp[:], g2ap, Act.Ln,
                                     bias=b_1eps[:], scale=-1.0)
                wn = wp.tile([W, HP], f32, tag=f"wn_{dx}_{dy}")
                if (dx, dy) > (0, 0):
                    g1ap = g1t[(dx, dy)][:, 2:2 + HP]
                else:
                    g1ap = g1s[(-dx, -dy)][:, 2 + dx:2 + dx + HP]
                nc.vector.tensor_sub(wn[:], g1ap, lnn[:])
                wnb = wn[:, None, :].broadcast_to((W, C, HP))
                lpb = lnp[:, None, :].broadcast_to((W, C, HP))
                xpap = xp_s[dy][:, :, 2 + dx:2 + dx + HP]
                if (dx, dy) in GP_TAPS:
                    if gp_first[0]:
                        nc.gpsimd.tensor_mul(accn2[:], wnb, xpap)
                        nc.gpsimd.tensor_mul(accp2[:], lpb, xpap)
                        gp_first[0] = False
                    else:
                        tn = mp.tile([W, C, HP], f32, tag="tng")
                        nc.gpsimd.tensor_mul(tn[:], wnb, xpap)
                        nc.gpsimd.tensor_add(accn2[:], accn2[:], tn[:])
                        tp = mp.tile([W, C, HP], f32, tag="tpg")
                        nc.gpsimd.tensor_mul(tp[:], lpb, xpap)
                        nc.gpsimd.tensor_add(accp2[:], accp2[:], tp[:])
                else:
                    tn = mp.tile([W, C, HP], f32, tag="tn")
                    nc.vector.tensor_mul(tn[:], wnb, xpap)
                    nc.vector.tensor_add(accn[:], accn[:], tn[:])
                    tp = mp.tile([W, C, HP], f32, tag="tp")
                    nc.vector.tensor_mul(tp[:], lpb, xpap)
                    nc.vector.tensor_add(accp[:], accp[:], tp[:])

            nc.vector.tensor_add(accn[:], accn[:], accn2[:])
            nc.vector.tensor_add(accp[:], accp[:], accp2[:])
            nc.scalar.activation(accn[:], accn[:], Act.Copy,
                                 bias=0.0, scale=COMPAT_CLSBD)
            nc.scalar.activation(accp[:], accp[:], Act.Copy,
                                 bias=0.0, scale=-COMPAT_CLSBD)
            nc.sync.dma_start(out=o_d[0], in_=accn[:])
            nc.sync.dma_start(out=o_d[1], in_=accp[:])
    nc.finalize()
    return nc


_last_results = None


def kernel(input, feats, clsbd_feats, label=None, **_ignored):
    global _last_results
    from concourse.bass_utils import run_bass_kernel_spmd

    x = np.asarray(input, np.float32)
    f = np.asarray(feats, np.float32)
    s = np.asarray(clsbd_feats, np.float32)

    xpad = np.zeros((B, C, H + 4, W), np.float32)
    xpad[:, :, 2:2 + H] = x
    fpad = np.full((B, D, H + 8, W), BIGPAD, np.float32)
    fpad[:, :, 4:4 + H] = f
    spad = np.zeros((B, H + 4, W), np.float32)
    spad[:, 2:2 + H] = s[:, 0]

    in_maps = []
    for i in range(8):
        b, half = i // 2, i % 2
        h0 = half * HP
        in_maps.append({
            "x": np.ascontiguousarray(
                xpad[b, :, h0:h0 + HE].transpose(2, 0, 1)),
            "f": np.ascontiguousarray(
                fpad[b, :, h0:h0 + FE].transpose(2, 0, 1)),
            "s": np.ascontiguousarray(spad[b, h0:h0 + HE].transpose(1, 0)),
        })

    if "nc" not in _cache:
        _cache["nc"] = _build()
    res = run_bass_kernel_spmd(_cache["nc"], in_maps, list(range(8)))
    _last_results = res

    out = np.empty((2, B, C, H, W), np.float32)
    for i in range(8):
        b, half = i // 2, i % 2
        h0 = half * HP
        out[:, b, :, h0:h0 + HP] = res.results[i]["out"].transpose(0, 2, 3, 1)
    return out


# revision 26
# speedup vs baseline: 1.2604x; 1.2604x over previous
"""ClsbdCRF message-passing kernel for 8 Trainium2 NeuronCores.

Sharding: core i handles batch b = i//2 and image-row half i%2 (64 output
rows each, with span-2 halos sliced host-side).  Per-core SBUF layout puts
W=128 on partitions and (C, H) on the free dimension, so the 5x5 stencil
becomes partition-offset (dy) + free-offset (dx) access patterns.

Math per core (fp32):
  pl   = 1 - ent/ln(C),  ent = -sum_c x ln(x+eps)
  xp   = x * pl
  g1_d = exp(-0.5 * ||f(x) - f(x+d)||^2)            (12 taps + mirrors + center)
  g2_t = ring-max propagation of unfolded clsbd map  (24 taps + center=0)
  w_neg_t = 2*g1_t - ln(g2_t+eps)          (x5 at the end)
  w_pos_t = ln(1 - g2_t + eps)             (x-5 at the end)
  msg[c,h,w] = sum_t w_t[h,w] * xp[c, (h,w)+t]

Boundary handling: H is zero-padded host-side (feats big-padded so the
pairwise gaussian underflows to exactly 0 out of image); W taps use
restricted partition ranges with pre-zeroed destination tiles.
"""

import math

import numpy as np

B, C, H, W, D = 4, 21, 128, 128, 5
SPAN = 2
EPS = 1e-5
HP = 64          # output rows per core
HE = HP + 4      # input / clsbd row extent (halo 2 each side)
FE = HP + 8      # feats row extent (halo 4 each side)
BIGPAD = 1000.0  # feats pad value; (BIGPAD-x)^2 makes exp() underflow to 0
COMPAT_PAIR = 10.0
COMPAT_CLSBD = 5.0

RING1 = [(-1, -1), (-1, 0), (-1, 1), (0, -1), (0, 1), (1, -1), (1, 0), (1, 1)]
RING2 = [(-2, -2), (-2, -1), (-2, 0), (-2, 1), (-2, 2), (-1, -2), (-1, 2),
         (0, -2), (0, 2), (1, -2), (1, 2), (2, -2), (2, -1), (2, 0), (2, 1),
         (2, 2)]
EXP1 = [0, 0, 1, 2, 2, 0, 2, 3, 4, 5, 7, 5, 5, 6, 7, 7]
EXP2 = [0, 1, 1, 1, 2, 3, 4, 3, 4, 3, 4, 5, 6, 6, 6, 7]
# taps whose pairwise gaussian is computed directly; mirrors are shifted reads
DIRTAPS = [(dx, dy) for dx in range(-SPAN, SPAN + 1)
           for dy in range(-SPAN, SPAN + 1) if (dx, dy) > (0, 0)]
ALLTAPS = [(dx, dy) for dx in range(-SPAN, SPAN + 1)
           for dy in range(-SPAN, SPAN + 1)]

GP_NTAPS = 8
_cache = {}


def _wrange(dy):
    return max(0, -dy), W - max(0, dy)


def _build():
    import concourse.bacc as bacc
    import concourse.mybir as mybir
    from concourse.tile import TileContext

    f32 = mybir.dt.float32
    Act = mybir.ActivationFunctionType
    Alu = mybir.AluOpType

    nc = bacc.Bacc()
    x_d = nc.declare_dram_parameter("x", [W, C, HE], f32, isOutput=False)
    f_d = nc.declare_dram_parameter("f", [W, D, FE], f32, isOutput=False)
    s_d = nc.declare_dram_parameter("s", [W, HE], f32, isOutput=False)
    o_d = nc.declare_dram_parameter("out", [2, W, C, HP], f32, isOutput=True)

    DYS = [-2, -1, 1, 2]

    # Pre-TileContext constants, covered by an all-engine barrier (same
    # pattern Bass.__init__ uses) so consumers never need a sync wait.
    def _const_sbuf(name, shape, val):
        t = nc.alloc_sbuf_tensor(name, shape, f32)
        nc.gpsimd.memset(t.ap(), val)
        return t.ap()

    zt = _const_sbuf("zt_const", [W, C, HE], 0.0)
    bpad = _const_sbuf("bpad_const", [W, D, FE], BIGPAD)
    b_eps = _const_sbuf("b_eps", [W, 1], EPS)
    b_ln2 = _const_sbuf("b_ln2", [W, 1], math.log(2.0))
    b_1eps = _const_sbuf("b_1eps", [W, 1], 1.0 + EPS)
    nc.const_aps.aps[(f32, EPS)] = b_eps

    # partition-shift matrices: S_dy[k, m] = 1 iff k = m + dy, so
    # (S_dy^T @ x)[m] = x[m+dy] with zero rows outside [0, W) — PE does
    # the partition shift straight into PSUM, no DMA descriptor storms.
    s_mat = {}
    for dy in (-2, -1, 1, 2):
        t = nc.alloc_sbuf_tensor(f"shift_{dy}", [W, W], f32)
        nc.gpsimd.memset(t.ap(), 0.0)
        nc.gpsimd.affine_select(
            out=t.ap(), in_=t.ap(), compare_op=mybir.AluOpType.not_equal,
            fill=1.0, base=-dy, pattern=[[-1, W]], channel_multiplier=1)
        s_mat[dy] = t.ap()
    nc.all_engine_barrier()

    # taps owned end-to-end by GpSimd (ready earliest: dy=0 needs no
    # shifted xp at all). DVE keeps the rest.
    GP_TAPS = [(1, 0), (2, 0), (-1, 0), (-2, 0)][:GP_NTAPS]

    with TileContext(nc) as tc:
        with (
            tc.tile_pool(name="io", bufs=1) as io,
            tc.tile_pool(name="g1p", bufs=1) as g1p,
            tc.tile_pool(name="g2p", bufs=1) as g2p,
            tc.tile_pool(name="wp", bufs=1) as wp,
            tc.tile_pool(name="lp", bufs=3) as lp,
            tc.tile_pool(name="mp", bufs=2) as mp,
            tc.tile_pool(name="sc", bufs=1) as sc,
            tc.tile_pool(name="scr", bufs=2) as scr,
            tc.tile_pool(name="dr", bufs=1, space="DRAM") as dr,
            tc.tile_pool(name="psp", bufs=2, space="PSUM") as psp,
        ):
            # ---- phase 0: all DRAM loads up front ----
            x_t = io.tile([W, C, HE], f32, tag="x")
            f_t = io.tile([W, D, FE], f32, tag="f")
            s_t = io.tile([W, HE], f32, tag="s")
            nc.sync.dma_start(out=x_t[:], in_=x_d[:])
            nc.sync.dma_start(out=f_t[:], in_=f_d[:])
            nc.sync.dma_start(out=s_t[:], in_=s_d[:])

            def _shift_load(pool, tag, shape, dram, padsrc, dy, eng=None):
                # dy-shifted copy straight from DRAM (one descriptor),
                # out-of-range partitions filled from a barrier-covered const
                eng = eng or nc.sync
                t = pool.tile(shape, f32, tag=tag)
                a, b = _wrange(dy)
                if a > 0:
                    eng.dma_start(out=t[:a], in_=padsrc[:a])
                if b < W:
                    eng.dma_start(out=t[b:], in_=padsrc[b:])
                eng.dma_start(out=t[a:b], in_=dram[a + dy:b + dy])
                return t

            f_s, s_s = {0: f_t}, {0: s_t}
            for dy in DYS:
                f_s[dy] = _shift_load(io, f"fs_{dy}", [W, D, FE], f_d, bpad,
                                      dy)
                s_s[dy] = _shift_load(io, f"ss_{dy}", [W, HE], s_d,
                                      zt[:, 0, :HE], dy)

            # ---- polarness ----
            lnx = sc.tile([W, C, HE], f32, tag="lnx")
            nc.scalar.activation(lnx[:], x_t[:], Act.Ln, bias=b_eps[:], scale=1.0)
            xl = sc.tile([W, C, HE], f32, tag="xl")
            nc.vector.tensor_mul(xl[:], x_t[:], lnx[:])
            ent = sc.tile([W, HE], f32, tag="ent")
            nc.vector.tensor_reduce(
                out=ent[:], in_=xl[:].rearrange("p c h -> p h c"),
                axis=mybir.AxisListType.X, op=Alu.add)
            pl = sc.tile([W, HE], f32, tag="pl")
            # ent holds sum_c x*ln(x+eps) = -entropy
            nc.scalar.activation(pl[:], ent[:], Act.Copy,
                                 bias=1.0, scale=1.0 / math.log(C))
            xp = io.tile([W, C, HE], f32, tag="xp")
            nc.vector.tensor_mul(
                xp[:], x_t[:], pl[:, None, :].broadcast_to((W, C, HE)))
            # xp dy-shifts: 3 matmuls per dy (<=512 fp32 moving-operand
            # cap; 512-f32 slices stay single-bank). 2 PSUM slots rotate
            # through the dy groups, so products must consume dy-major.
            xp_flat = xp[:].rearrange("p c h -> p (c h)")
            FSL = [(0, 512), (512, 1024), (1024, C * HE)]
            xp_s = {0: xp}
            for dy in (-1, 1, -2, 2):
                t = psp.tile([W, C, HE], f32, tag="xps")
                tf = t[:].rearrange("p c h -> p (c h)")
                for (n0, n1) in FSL:
                    nc.tensor.matmul(tf[:, n0:n1], s_mat[dy],
                                     xp_flat[:, n0:n1], start=True, stop=True)
                xp_s[dy] = t

            # ---- pairwise gaussian (12 direct taps; value stored = 2*g1) ----
            # BIGPAD-shifted feats make out-of-image taps underflow to 0.
            g1t = {}
            g1d = {}
            for (dx, dy) in DIRTAPS:
                g1 = g1p.tile([W, HE], f32, tag=f"g1_{dx}_{dy}")
                diff = scr.tile([W, D, HE], f32, tag="diff")
                nc.vector.tensor_sub(
                    diff[:], f_t[:, :, 2:2 + HE],
                    f_s[dy][:, :, 2 + dx:2 + dx + HE])
                sq = scr.tile([W, D, HE], f32, tag="sq")
                nc.scalar.square(sq[:], diff[:])
                ssum = scr.tile([W, HE], f32, tag="ssum")
                nc.vector.tensor_reduce(
                    out=ssum[:], in_=sq[:].rearrange("p d h -> p h d"),
                    axis=mybir.AxisListType.X, op=Alu.add)
                nc.scalar.activation(g1[:], ssum[:], Act.Exp,
                                     bias=b_ln2[:], scale=-0.5)
                g1t[(dx, dy)] = g1
                if dy != 0:
                    gd = dr.tile([W, HE], f32, tag=f"g1d_{dx}_{dy}")
                    nc.scalar.dma_start(out=gd[:], in_=g1[:])
                    g1d[(dx, dy)] = gd
            # dy-shifted mirror copies via DRAM roundtrip
            g1s = {}
            for (dx, dy) in DIRTAPS:
                if dy == 0:
                    g1s[(dx, dy)] = g1t[(dx, dy)]
                    continue
                a, b = _wrange(-dy)
                t = g1p.tile([W, HE], f32, tag=f"g1s_{dx}_{dy}")
                if a > 0:
                    nc.scalar.dma_start(out=t[:a], in_=zt[:a, 0, :HE])
                if b < W:
                    nc.scalar.dma_start(out=t[b:], in_=zt[b:, 0, :HE])
                nc.scalar.dma_start(out=t[a:b],
                                    in_=g1d[(dx, dy)][a - dy:b - dy])
                g1s[(dx, dy)] = t

            # ---- clsbd gaussian: ring max propagation ----
            tmp1 = [s_s[dy][:, 2 + dx:2 + dx + HP] for (dx, dy) in RING1]
            g2t = {t: tmp1[j] for j, t in enumerate(RING1)}
            for k, (dx, dy) in enumerate(RING2):
                t2 = g2p.tile([W, HP], f32, tag=f"t2_{k}")
                nc.vector.tensor_max(t2[:], tmp1[EXP1[k]], tmp1[EXP2[k]])
                nc.vector.tensor_max(
                    t2[:], t2[:], s_s[dy][:, 2 + dx:2 + dx + HP])
                g2t[(dx, dy)] = t2

            # ---- weights for all 24 taps ----
            NT = [t for t in ALLTAPS if t != (0, 0)]
            # GpSimd-owned taps first in emission so its chain starts early
            DYRANK = {0: -1, -1: 0, 1: 1, -2: 2, 2: 3}
            NT.sort(key=lambda t: (t not in GP_TAPS, DYRANK[t[1]]))
            wns, lnps = {}, {}
            for (dx, dy) in NT:
                g2 = g2t[(dx, dy)]
                g2ap = g2[:] if hasattr(g2, "tag") else g2
                lnn = lp.tile([W, HP], f32, tag="lnn")
                nc.scalar.activation(lnn[:], g2ap, Act.Ln, bias=b_eps[:],
                                     scale=1.0)
                lnp = wp.tile([W, HP], f32, tag=f"lnp_{dx}_{dy}")
                nc.scalar.activation(lnp[:], g2ap, Act.Ln,
                                     bias=b_1eps[:], scale=-1.0)
                wn = wp.tile([W, HP], f32, tag=f"wn_{dx}_{dy}")
                if (dx, dy) > (0, 0):
                    g1ap = g1t[(dx, dy)][:, 2:2 + HP]
                else:
                    g1ap = g1s[(-dx, -dy)][:, 2 + dx:2 + dx + HP]
                nc.vector.tensor_sub(wn[:], g1ap, lnn[:])
                wns[(dx, dy)] = wn
                lnps[(dx, dy)] = lnp

            # ---- products + accumulation ----
            accn = io.tile([W, C, HP], f32, tag="accn")
            accp = io.tile([W, C, HP], f32, tag="accp")
            accn2 = io.tile([W, C, HP], f32, tag="accn2")
            accp2 = io.tile([W, C, HP], f32, tag="accp2")
            xp_c = xp[:, :, 2:2 + HP]
            nc.vector.tensor_scalar_mul(accn[:], xp_c, 2.0 - math.log(EPS))
            nc.vector.tensor_scalar_mul(accp[:], xp_c, math.log(1.0 + EPS))
            gp_first = [True]
            for (dx, dy) in NT:
                wnb = wns[(dx, dy)][:, None, :].broadcast_to((W, C, HP))
                lpb = lnps[(dx, dy)][:, None, :].broadcast_to((W, C, HP))
                xpap = xp_s[dy][:, :, 2 + dx:2 + dx + HP]
                if (dx, dy) in GP_TAPS:
                    if gp_first[0]:
                        nc.gpsimd.tensor_mul(accn2[:], wnb, xpap)
                        nc.gpsimd.tensor_mul(accp2[:], lpb, xpap)
                        gp_first[0] = False
                    else:
                        tn = mp.tile([W, C, HP], f32, tag="tng")
                        nc.gpsimd.tensor_mul(tn[:], wnb, xpap)
                        nc.gpsimd.tensor_add(accn2[:], accn2[:], tn[:])
                        tp = mp.tile([W, C, HP], f32, tag="tpg")
                        nc.gpsimd.tensor_mul(tp[:], lpb, xpap)
                        nc.gpsimd.tensor_add(accp2[:], accp2[:], tp[:])
                else:
                    tn = mp.tile([W, C, HP], f32, tag="tn")
                    nc.vector.tensor_mul(tn[:], wnb, xpap)
                    nc.vector.tensor_add(accn[:], accn[:], tn[:])
                    tp = mp.tile([W, C, HP], f32, tag="tp")
                    nc.vector.tensor_mul(tp[:], lpb, xpap)
                    nc.vector.tensor_add(accp[:], accp[:], tp[:])

            nc.vector.tensor_add(accn[:], accn[:], accn2[:])
            nc.vector.tensor_add(accp[:], accp[:], accp2[:])
            nc.scalar.activation(accn[:], accn[:], Act.Copy,
                                 bias=0.0, scale=COMPAT_CLSBD)
            nc.scalar.activation(accp[:], accp[:], Act.Copy,
                                 bias=0.0, scale=-COMPAT_CLSBD)
            nc.sync.dma_start(out=o_d[0], in_=accn[:])
            nc.sync.dma_start(out=o_d[1], in_=accp[:])
    nc.finalize()
    return nc


_last_results = None


def kernel(input, feats, clsbd_feats, label=None, **_ignored):
    global _last_results
    from concourse.bass_utils import run_bass_kernel_spmd

    x = np.asarray(input, np.float32)
    f = np.asarray(feats, np.float32)
    s = np.asarray(clsbd_feats, np.float32)

    xpad = np.zeros((B, C, H + 4, W), np.float32)
    xpad[:, :, 2:2 + H] = x
    fpad = np.full((B, D, H + 8, W), BIGPAD, np.float32)
    fpad[:, :, 4:4 + H] = f
    spad = np.zeros((B, H + 4, W), np.float32)
    spad[:, 2:2 + H] = s[:, 0]

    in_maps = []
    for i in range(8):
        b, half = i // 2, i % 2
        h0 = half * HP
        in_maps.append({
            "x": np.ascontiguousarray(
                xpad[b, :, h0:h0 + HE].transpose(2, 0, 1)),
            "f": np.ascontiguousarray(
                fpad[b, :, h0:h0 + FE].transpose(2, 0, 1)),
            "s": np.ascontiguousarray(spad[b, h0:h0 + HE].transpose(1, 0)),
        })

    if "nc" not in _cache:
        _cache["nc"] = _build()
    res = run_bass_kernel_spmd(_cache["nc"], in_maps, list(range(8)))
    _last_results = res

    out = np.empty((2, B, C, H, W), np.float32)
    for i in range(8):
        b, half = i // 2, i % 2
        h0 = half * HP
        out[:, b, :, h0:h0 + HP] = res.results[i]["out"].transpose(0, 2, 3, 1)
    return out
